# revision 38
# baseline (speedup 1.0000x reference)
"""Trainium2 Bass kernel for nn_ContrastiveLoss (N=8192, D=1024, 751 ids).

loss = (1/N) * sum_ij [ same(i,j) & sim<1 -> (1-sim) ; diff(i,j) & sim>0.3 -> sim ]
with sim = X @ X.T.

Strategy (8 NeuronCores):
  * Host: sort rows by label (loss is permutation invariant); same-label
    pairs then live within +-63 of the diagonal (max class count <= 64).
    Quantize X to fp8 e4m3 -> PE runs DoubleRow matmuls (K=256/pass).
  * sim is symmetric: orient each unordered block pair {a,b} of the 16
    512-row blocks toward head a if (b-a) mod 16 in 1..8 (a<8) / 1..7.
    Core c takes heads c (9 items) and c+8 (8 items) -> 17 items/core,
    all 136 pairs covered once. Off-diagonal pairs weigh 2x.
  * Slot-uniform program: core c's DRAM holds the 16 blocks rotated by
    c (slot s = block (c+s) mod 16), so one program serves all cores.
    Each block is DMA'd once (fp8, 512KB) and stays resident in SBUF;
    items read lhs from slot 0/8, rhs from slot i -> ~8.5MB DMA/core.
  * Per item: 2 [128,1024] two-bank PSUM tiles (2 m-subtiles each),
    4 DoubleRow matmuls per subtile.  Unmasked sums need no label mask:
      sum_j sim*1[sim>0.3] = sum relu(sim-0.3) + 0.3*count(sim>0.3),
    ScalarE Relu (fp16 out) with fused accum_out; the count comes from
    VectorE is_gt on the fp16 relu output (2x DVE rate, and PSUM then
    has a single reader).
  * Band correction (circular 256-wide windows around the diagonal) on
    rows of blocks c and c+8 (slots 0/8): for same-label pairs subtract
    the neg term and add relu(1-sim), label-equality masked.  Windows
    slice the *same* resident tiles, so the sim values cancel bitwise.
    Wrapped window columns are killed by the label mask.  Band items run
    in the first half so their DVE chains drain mid-kernel.
  * Junk warmup matmuls at t=0 open the HAM clock gate (2.4GHz) while
    the first block DMA lands.
  * Host: gather per-item partial sums, weight (1x diag / 2x off-diag),
    reduce in float64.
"""

import sys

for _p in ("/opt/trn_rl_repo",):
    if _p not in sys.path:
        sys.path.append(_p)

import numpy as np
import ml_dtypes

import concourse.bass as bass
import concourse.mybir as mybir
import concourse.tile as tile
from concourse import bacc
from concourse.bass_utils import run_bass_kernel_spmd

N = 8192           # rows
D = 1024           # feature dim
NCORES = 8
B = 512            # block size
NB = N // B        # 16 blocks/slots
NIT = 17           # items (block pairs) per core
MS = B // 128      # m-subtiles per item = 4
KT2 = D // 256     # DoubleRow contraction chunks = 4
MT = 8             # band row-tiles per core (2 groups x 4)
BW = 256           # band window width
MARGIN = 0.3

f8 = mybir.dt.float8e4
f16 = mybir.dt.float16
f32 = mybir.dt.float32

# output columns: per-half-item relu sums [0,34), counts [34,68),
# band corr [68,76), warmup junk [76]
MH = 2                  # m-subtile pairs per item ([128,1024] PSUM tiles)
C_R = 0
C_C = NIT * MH          # 34
C_B = 2 * NIT * MH      # 68
C_OUT = C_B + MT + 1    # 77

_CACHE = {}


def _band_segments(gi, j):
    """Window segments (slot, col0, width) for band row-tile j of group gi.

    Window = circular [S*512 + j*128 - 64, +256) in slot-column space.
    """
    S = 8 * gi
    if j == 0:
        return [((S - 1) % NB, 448, 64), (S, 0, 192)]
    if j == 3:
        return [(S, 320, 192), ((S + 1) % NB, 0, 64)]
    return [(S, j * 128 - 64, 256)]


def _build_program():
    nc = bacc.Bacc("TRN2", target_bir_lowering=False, debug=False,
                   num_devices=NCORES)

    # [slot, partition, (g,h), col]: slot s holds block (c+s)%16 of X^T,
    # k-row (2g+h)*128+p, quantized to fp8 e4m3.
    xq = nc.dram_tensor("xq", [NB, 128, 2 * KT2, B], f8, kind="ExternalInput")
    eqm = nc.dram_tensor("eqm", [128, MT * BW], f16, kind="ExternalInput")
    outp = nc.dram_tensor("out", [128, C_OUT], f32, kind="ExternalOutput")

    Relu = mybir.ActivationFunctionType.Relu
    Sign = mybir.ActivationFunctionType.Sign
    Op = mybir.AluOpType
    DR = mybir.MatmulPerfMode.DoubleRow

    with tile.TileContext(nc) as tc:
        with (
            tc.tile_pool(name="persist", bufs=1) as persist,
            tc.tile_pool(name="scr", bufs=3) as scr,
            tc.tile_pool(name="band", bufs=3) as bandp,
            tc.tile_pool(name="psum_m", bufs=3, space="PSUM") as psum_m,
            tc.tile_pool(name="psum_b", bufs=2, space="PSUM") as psum_b,
        ):
            T = [persist.tile([128, 2 * KT2, B], f8, name=f"blk{s}")
                 for s in range(NB)]
            # contiguous band-window tiles, SBUF->SBUF copied from the
            # resident slot tiles (same bytes -> sims cancel bitwise);
            # one 4-matmul chain per band item instead of split segments
            W = [persist.tile([128, 2 * KT2, BW], f8, name=f"win{b}")
                 for b in range(MT)]
            eqm_sb = persist.tile([128, MT * BW], f16, name="eqm")
            stats = persist.tile([128, C_OUT], f32, name="stats")
            bias_m = persist.tile([128, 1], f32, name="bias_m")
            nc.vector.memset(bias_m[:], -MARGIN)

            # HAM warmup: junk matmuls keep the PE busy while the first
            # block DMA lands, so the clock gate opens before real work.
            wm = persist.tile([128, 2, 128], f8, name="wm")
            nc.vector.memset(wm[:], 0.125)
            wps = psum_m.tile([128, 2 * B], f32, name="mm")
            for w in range(24):
                nc.tensor.matmul(
                    wps[:, :128], wm[:], wm[:],
                    start=(w == 0), stop=(w == 23), perf_mode=DR)
            wsr = scr.tile([128, 128], f16, name="wsr")
            nc.scalar.activation(
                wsr[:], wps[:, :128], Relu,
                accum_out=stats[:, C_OUT - 1:C_OUT])

            def main_item(i):
                L = T[0] if i <= 8 else T[8]
                R = T[i] if i <= 8 else T[i - 1]
                for h in range(MH):
                    ps = psum_m.tile([128, 2 * B], f32, name="mm")
                    for m2 in range(2):
                        m = 2 * h + m2
                        pj = ps[:, m2 * B:(m2 + 1) * B]
                        for g in range(KT2):
                            nc.tensor.matmul(
                                pj,
                                L[:, 2 * g:2 * g + 2, m * 128:(m + 1) * 128],
                                R[:, 2 * g:2 * g + 2, :],
                                start=(g == 0), stop=(g == KT2 - 1),
                                perf_mode=DR,
                            )
                    col = i * MH + h
                    sr = scr.tile([128, 2 * B], f16, name="sr")
                    nc.scalar.activation(
                        sr[:], ps[:], Relu, bias=bias_m[:],
                        accum_out=stats[:, C_R + col:C_R + col + 1])
                    sc = scr.tile([128, 2 * B], f16, name="sc")
                    nc.vector.tensor_scalar(
                        sc[:], ps[:], MARGIN, None, op0=Op.is_gt,
                        op1=Op.add,
                        accum_out=stats[:, C_C + col:C_C + col + 1])

            def band_item(gi, j):
                """One [128 x 256] diagonal-window correction."""
                S = 8 * gi
                bj = gi * 4 + j
                ps = psum_b.tile([128, BW], f32, name="bb")
                for g in range(KT2):
                    nc.tensor.matmul(
                        ps[:],
                        T[S][:, 2 * g:2 * g + 2, j * 128:(j + 1) * 128],
                        W[bj][:, 2 * g:2 * g + 2, :],
                        start=(g == 0), stop=(g == KT2 - 1),
                        perf_mode=DR,
                    )
                pos = bandp.tile([128, BW], f32, name="pos")
                rb = bandp.tile([128, BW], f32, name="rb")
                sg = bandp.tile([128, BW], f32, name="sg")
                # pos = relu(1 - s);  rb = relu(s - 0.3);  sg = sign(s - 0.3)
                # neg = rb + 0.3*1[s>0.3] = rb + 0.15*sg + 0.15  (a.e.)
                nc.scalar.activation(pos[:], ps[:], Relu, bias=1.0, scale=-1.0)
                nc.scalar.activation(rb[:], ps[:], Relu, bias=bias_m[:])
                nc.scalar.activation(sg[:], ps[:], Sign, bias=bias_m[:])
                # corr (negated) = eq * (neg - pos); host subtracts it
                a = bandp.tile([128, BW], f32, name="a")
                nc.vector.scalar_tensor_tensor(
                    a[:], sg[:], 0.15, pos[:], op0=Op.mult,
                    op1=Op.subtract)
                b = bandp.tile([128, BW], f32, name="b")
                nc.vector.tensor_tensor(b[:], a[:], rb[:], op=Op.add)
                crr = bandp.tile([128, BW], f32, name="crr")
                nc.vector.scalar_tensor_tensor(
                    crr[:], b[:], 0.15, eqm_sb[:, bj * BW:(bj + 1) * BW],
                    op0=Op.add, op1=Op.mult,
                    accum_out=stats[:, C_B + bj:C_B + bj + 1])

            # DMA slot order: band-needed slots (15, 7, 8) pulled early so
            # all band items can run in the first half; mains stay fed.
            dma_order = [0, 1, 2, 15, 3, 7, 8, 4, 5, 9, 6, 10, 11, 12, 13, 14]
            # band items early so their DVE chains drain mid-kernel, not
            # as a serialized tail behind the last main items
            bands_after = {
                0: [(0, 1)], 1: [(0, 2)], 2: [(0, 0)], 3: [(0, 3)],
                4: [(1, 0)], 5: [(1, 1)], 6: [(1, 2)], 7: [(1, 3)],
            }

            # T0 split in half so the first matmuls start sooner
            nc.sync.dma_start(T[0][:, 0:KT2, :], xq[0][:, 0:KT2, :])
            nc.sync.dma_start(T[0][:, KT2:, :], xq[0][:, KT2:, :])
            for p in range(1, 4):
                nc.sync.dma_start(T[dma_order[p]][:], xq[dma_order[p]])
            nc.sync.dma_start(eqm_sb[:], eqm[:])

            def win_copy(bj):
                woff = 0
                for (sl, c0, w) in _band_segments(bj // 4, bj % 4):
                    nc.sync.dma_start(W[bj][:, :, woff:woff + w],
                                      T[sl][:, :, c0:c0 + w])
                    woff += w

            # group-0 windows source T15/T0/T1 (all issued above)
            for bj in (1, 2, 0, 3):
                win_copy(bj)
            wins_after = {1: (5, 6), 2: (4,), 3: (7,)}
            for i in range(NIT):
                for p in (2 * i + 4, 2 * i + 5):
                    if p < NB:
                        nc.sync.dma_start(T[dma_order[p]][:],
                                          xq[dma_order[p]])
                for bj in wins_after.get(i, ()):
                    win_copy(bj)
                main_item(i)
                for (gi, j) in bands_after.get(i, ()):
                    band_item(gi, j)

            nc.sync.dma_start(outp[:], stats[:])

    nc.compile()
    return nc


def _prepare_in_maps(X, t):
    perm = np.argsort(t, kind="stable")
    Xs = X[perm]
    ts = t[perm].astype(np.int64)
    counts = np.bincount(ts)
    maxc = int(counts.max()) if counts.size else 0
    assert maxc <= 64, f"class count {maxc} exceeds band half-width 64"
    XT = np.ascontiguousarray(Xs.astype(ml_dtypes.float8_e4m3).T)  # [D, N]
    # [b, p, (g,h), col]
    blocks = np.ascontiguousarray(
        XT.reshape(KT2, 2, 128, NB, B).transpose(3, 2, 0, 1, 4)
    ).reshape(NB, 128, 2 * KT2, B)
    tsf = ts.astype(np.float16)  # exact for ids < 2048

    in_maps = []
    for c in range(NCORES):
        order = [(c + s) % NB for s in range(NB)]
        xqc = np.ascontiguousarray(blocks[order])
        eq = np.empty((128, MT * BW), np.float16)
        for gi in range(2):
            base = (c + 8 * gi) % NB
            for j in range(4):
                bj = gi * 4 + j
                r0 = base * B + j * 128
                idx = (np.arange(BW) + r0 - 64) % N
                eq[:, bj * BW:(bj + 1) * BW] = (
                    ts[r0:r0 + 128, None] == ts[idx][None, :])
        in_maps.append({"xq": xqc, "eqm": eq})
    return in_maps


# item weights: diag items (slots 0 and 8 vs themselves) 1x, rest 2x
_W_ITEM = np.array([1.0] + [2.0] * 8 + [1.0] + [2.0] * 7, np.float64)


def _reduce_outputs(results):
    tot = 0.0
    for c in range(NCORES):
        o = np.asarray(results[c]["out"], np.float64)
        r_items = o[:, C_R:C_C].sum(axis=0).reshape(NIT, MH).sum(axis=1)
        c_items = o[:, C_C:C_B].sum(axis=0).reshape(NIT, MH).sum(axis=1)
        neg_items = r_items + MARGIN * c_items
        tot += float((_W_ITEM * neg_items).sum())
        # band cols hold eq*(neg - pos); subtract
        tot -= float(o[:, C_B:C_B + MT].sum())
    return np.float32(tot / float(N))


def kernel(inputs, targets, _trace=False, _tmpdir=None):
    X = np.asarray(inputs, dtype=np.float32)
    t = np.asarray(targets)
    assert X.shape == (N, D)

    if "nc" not in _CACHE:
        _CACHE["nc"] = _build_program()
    nc = _CACHE["nc"]

    in_maps = _prepare_in_maps(X, t)
    res = run_bass_kernel_spmd(
        nc, in_maps, list(range(NCORES)), trace=_trace, tmpdir=_tmpdir)
    loss = _reduce_outputs(res.results)
    if _trace:
        return loss, res
    return loss


# revision 39
# speedup vs baseline: 1.0338x; 1.0338x over previous
"""Trainium2 Bass kernel for nn_ContrastiveLoss (N=8192, D=1024, 751 ids).

loss = (1/N) * sum_ij [ same(i,j) & sim<1 -> (1-sim) ; diff(i,j) & sim>0.3 -> sim ]
with sim = X @ X.T.

Strategy (8 NeuronCores):
  * Host: sort rows by label (loss is permutation invariant); same-label
    pairs then live within +-63 of the diagonal (max class count <= 64).
    Quantize X to fp8 e4m3 -> PE runs DoubleRow matmuls (K=256/pass).
  * sim is symmetric: orient each unordered block pair {a,b} of the 16
    512-row blocks toward head a if (b-a) mod 16 in 1..8 (a<8) / 1..7.
    Core c takes heads c (9 items) and c+8 (8 items) -> 17 items/core,
    all 136 pairs covered once. Off-diagonal pairs weigh 2x.
  * Slot-uniform program: core c's DRAM holds the 16 blocks rotated by
    c (slot s = block (c+s) mod 16), so one program serves all cores.
    Each block is DMA'd once (fp8, 512KB) and stays resident in SBUF;
    items read lhs from slot 0/8, rhs from slot i -> ~8.5MB DMA/core.
  * Per item: 2 [128,1024] two-bank PSUM tiles (2 m-subtiles each),
    4 DoubleRow matmuls per subtile.  Unmasked sums need no label mask:
      sum_j sim*1[sim>0.3] = sum relu(sim-0.3) + 0.3*count(sim>0.3),
    ScalarE Relu (fp16 out) with fused accum_out; the count comes from
    VectorE is_gt on the fp16 relu output (2x DVE rate, and PSUM then
    has a single reader).
  * Band correction (circular 256-wide windows around the diagonal) on
    rows of blocks c and c+8 (slots 0/8): for same-label pairs subtract
    the neg term and add relu(1-sim), label-equality masked.  Windows
    slice the *same* resident tiles, so the sim values cancel bitwise.
    Wrapped window columns are killed by the label mask.  Band items run
    in the first half so their DVE chains drain mid-kernel.
  * Junk warmup matmuls at t=0 open the HAM clock gate (2.4GHz) while
    the first block DMA lands.
  * Host: gather per-item partial sums, weight (1x diag / 2x off-diag),
    reduce in float64.
"""

import sys

for _p in ("/opt/trn_rl_repo",):
    if _p not in sys.path:
        sys.path.append(_p)

import numpy as np
import ml_dtypes

import concourse.bass as bass
import concourse.mybir as mybir
import concourse.tile as tile
from concourse import bacc
from concourse.bass_utils import run_bass_kernel_spmd

N = 8192           # rows
D = 1024           # feature dim
NCORES = 8
B = 512            # block size
NB = N // B        # 16 blocks/slots
NIT = 17           # items (block pairs) per core
MS = B // 128      # m-subtiles per item = 4
KT2 = D // 256     # DoubleRow contraction chunks = 4
MT = 8             # band row-tiles per core (2 groups x 4)
BW = 256           # band window width
MARGIN = 0.3

f8 = mybir.dt.float8e4
f16 = mybir.dt.float16
f32 = mybir.dt.float32

# output columns: per-half-item relu sums [0,34), counts [34,68),
# band corr [68,76), warmup junk [76]
MH = 2                  # m-subtile pairs per item ([128,1024] PSUM tiles)
C_R = 0
C_C = NIT * MH          # 34
C_B = 2 * NIT * MH      # 68
C_OUT = C_B + MT + 1    # 77

_CACHE = {}


def _band_segments(gi, j):
    """Window segments (slot, col0, width) for band row-tile j of group gi.

    Window = circular [S*512 + j*128 - 64, +256) in slot-column space.
    """
    S = 8 * gi
    if j == 0:
        return [((S - 1) % NB, 448, 64), (S, 0, 192)]
    if j == 3:
        return [(S, 320, 192), ((S + 1) % NB, 0, 64)]
    return [(S, j * 128 - 64, 256)]


def _build_program():
    nc = bacc.Bacc("TRN2", target_bir_lowering=False, debug=False,
                   num_devices=NCORES)

    # [slot, partition, (g,h), col]: slot s holds block (c+s)%16 of X^T,
    # k-row (2g+h)*128+p, quantized to fp8 e4m3.
    xq = nc.dram_tensor("xq", [NB, 128, 2 * KT2, B], f8, kind="ExternalInput")
    eqm = nc.dram_tensor("eqm", [128, MT * BW], f16, kind="ExternalInput")
    outp = nc.dram_tensor("out", [128, C_OUT], f32, kind="ExternalOutput")

    Relu = mybir.ActivationFunctionType.Relu
    Sign = mybir.ActivationFunctionType.Sign
    Op = mybir.AluOpType
    DR = mybir.MatmulPerfMode.DoubleRow

    with tile.TileContext(nc) as tc:
        with (
            tc.tile_pool(name="persist", bufs=1) as persist,
            tc.tile_pool(name="scr", bufs=3) as scr,
            tc.tile_pool(name="band", bufs=3) as bandp,
            tc.tile_pool(name="psum_m", bufs=3, space="PSUM") as psum_m,
            tc.tile_pool(name="psum_b", bufs=2, space="PSUM") as psum_b,
        ):
            T = [persist.tile([128, 2 * KT2, B], f8, name=f"blk{s}")
                 for s in range(NB)]
            eqm_sb = persist.tile([128, MT * BW], f16, name="eqm")
            stats = persist.tile([128, C_OUT], f32, name="stats")
            bias_m = persist.tile([128, 1], f32, name="bias_m")
            nc.vector.memset(bias_m[:], -MARGIN)

            # HAM warmup: junk matmuls keep the PE busy while the first
            # block DMA lands, so the clock gate opens before real work.
            wm = persist.tile([128, 2, 128], f8, name="wm")
            nc.vector.memset(wm[:], 0.125)
            wps = psum_m.tile([128, 2 * B], f32, name="mm")
            for w in range(24):
                nc.tensor.matmul(
                    wps[:, :128], wm[:], wm[:],
                    start=(w == 0), stop=(w == 23), perf_mode=DR)
            wsr = scr.tile([128, 128], f16, name="wsr")
            nc.scalar.activation(
                wsr[:], wps[:, :128], Relu,
                accum_out=stats[:, C_OUT - 1:C_OUT])

            def main_item(i):
                L = T[0] if i <= 8 else T[8]
                R = T[i] if i <= 8 else T[i - 1]
                for h in range(MH):
                    ps = psum_m.tile([128, 2 * B], f32, name="mm")
                    for m2 in range(2):
                        m = 2 * h + m2
                        pj = ps[:, m2 * B:(m2 + 1) * B]
                        for g in range(KT2):
                            nc.tensor.matmul(
                                pj,
                                L[:, 2 * g:2 * g + 2, m * 128:(m + 1) * 128],
                                R[:, 2 * g:2 * g + 2, :],
                                start=(g == 0), stop=(g == KT2 - 1),
                                perf_mode=DR,
                            )
                    col = i * MH + h
                    sr = scr.tile([128, 2 * B], f16, name="sr")
                    nc.scalar.activation(
                        sr[:], ps[:], Relu, bias=bias_m[:],
                        accum_out=stats[:, C_R + col:C_R + col + 1])
                    sc = scr.tile([128, 2 * B], f16, name="sc")
                    nc.vector.tensor_scalar(
                        sc[:], ps[:], MARGIN, None, op0=Op.is_gt,
                        op1=Op.add,
                        accum_out=stats[:, C_C + col:C_C + col + 1])

            def band_item(gi, j):
                """One [128 x 256] diagonal-window correction."""
                S = 8 * gi
                bj = gi * 4 + j
                ps = psum_b.tile([128, BW], f32, name="bb")
                off = 0
                for (sl, c0, w) in _band_segments(gi, j):
                    pj = ps[:, off:off + w]
                    for g in range(KT2):
                        nc.tensor.matmul(
                            pj,
                            T[S][:, 2 * g:2 * g + 2, j * 128:(j + 1) * 128],
                            T[sl][:, 2 * g:2 * g + 2, c0:c0 + w],
                            start=(g == 0), stop=(g == KT2 - 1),
                            perf_mode=DR,
                        )
                    off += w
                pos = bandp.tile([128, BW], f32, name="pos")
                rb = bandp.tile([128, BW], f32, name="rb")
                sg = bandp.tile([128, BW], f32, name="sg")
                # pos = relu(1 - s);  rb = relu(s - 0.3);  sg = sign(s - 0.3)
                # neg = rb + 0.3*1[s>0.3] = rb + 0.15*sg + 0.15  (a.e.)
                nc.scalar.activation(pos[:], ps[:], Relu, bias=1.0, scale=-1.0)
                nc.scalar.activation(rb[:], ps[:], Relu, bias=bias_m[:])
                nc.scalar.activation(sg[:], ps[:], Sign, bias=bias_m[:])
                # corr (negated) = eq * (neg - pos); host subtracts it
                a = bandp.tile([128, BW], f32, name="a")
                nc.vector.scalar_tensor_tensor(
                    a[:], sg[:], 0.15, pos[:], op0=Op.mult,
                    op1=Op.subtract)
                b = bandp.tile([128, BW], f32, name="b")
                nc.vector.tensor_tensor(b[:], a[:], rb[:], op=Op.add)
                crr = bandp.tile([128, BW], f32, name="crr")
                nc.vector.scalar_tensor_tensor(
                    crr[:], b[:], 0.15, eqm_sb[:, bj * BW:(bj + 1) * BW],
                    op0=Op.add, op1=Op.mult,
                    accum_out=stats[:, C_B + bj:C_B + bj + 1])

            # DMA slot order: band-needed slots (15, 7, 8) pulled early so
            # all band items can run in the first half; mains stay fed.
            dma_order = [0, 1, 2, 15, 3, 7, 8, 4, 5, 9, 6, 10, 11, 12, 13, 14]
            # band items early so their DVE chains drain mid-kernel, not
            # as a serialized tail behind the last main items
            bands_after = {
                0: [(0, 1)], 1: [(0, 2)], 2: [(0, 0)], 3: [(0, 3)],
                4: [(1, 0)], 5: [(1, 1)], 6: [(1, 2)], 7: [(1, 3)],
            }

            # T0 split in half so the first matmuls start sooner
            nc.sync.dma_start(T[0][:, 0:KT2, :], xq[0][:, 0:KT2, :])
            nc.sync.dma_start(T[0][:, KT2:, :], xq[0][:, KT2:, :])
            for p in range(1, 4):
                nc.sync.dma_start(T[dma_order[p]][:], xq[dma_order[p]])
            nc.sync.dma_start(eqm_sb[:], eqm[:])
            for i in range(NIT):
                for p in (2 * i + 4, 2 * i + 5):
                    if p < NB:
                        nc.sync.dma_start(T[dma_order[p]][:],
                                          xq[dma_order[p]])
                main_item(i)
                for (gi, j) in bands_after.get(i, ()):
                    band_item(gi, j)

            nc.sync.dma_start(outp[:], stats[:])

    nc.compile()
    return nc


def _prepare_in_maps(X, t):
    perm = np.argsort(t, kind="stable")
    Xs = X[perm]
    ts = t[perm].astype(np.int64)
    counts = np.bincount(ts)
    maxc = int(counts.max()) if counts.size else 0
    assert maxc <= 64, f"class count {maxc} exceeds band half-width 64"
    XT = np.ascontiguousarray(Xs.astype(ml_dtypes.float8_e4m3).T)  # [D, N]
    # [b, p, (g,h), col]
    blocks = np.ascontiguousarray(
        XT.reshape(KT2, 2, 128, NB, B).transpose(3, 2, 0, 1, 4)
    ).reshape(NB, 128, 2 * KT2, B)
    tsf = ts.astype(np.float16)  # exact for ids < 2048

    in_maps = []
    for c in range(NCORES):
        order = [(c + s) % NB for s in range(NB)]
        xqc = np.ascontiguousarray(blocks[order])
        eq = np.empty((128, MT * BW), np.float16)
        for gi in range(2):
            base = (c + 8 * gi) % NB
            for j in range(4):
                bj = gi * 4 + j
                r0 = base * B + j * 128
                idx = (np.arange(BW) + r0 - 64) % N
                eq[:, bj * BW:(bj + 1) * BW] = (
                    ts[r0:r0 + 128, None] == ts[idx][None, :])
        in_maps.append({"xq": xqc, "eqm": eq})
    return in_maps


# item weights: diag items (slots 0 and 8 vs themselves) 1x, rest 2x
_W_ITEM = np.array([1.0] + [2.0] * 8 + [1.0] + [2.0] * 7, np.float64)


def _reduce_outputs(results):
    tot = 0.0
    for c in range(NCORES):
        o = np.asarray(results[c]["out"], np.float64)
        r_items = o[:, C_R:C_C].sum(axis=0).reshape(NIT, MH).sum(axis=1)
        c_items = o[:, C_C:C_B].sum(axis=0).reshape(NIT, MH).sum(axis=1)
        neg_items = r_items + MARGIN * c_items
        tot += float((_W_ITEM * neg_items).sum())
        # band cols hold eq*(neg - pos); subtract
        tot -= float(o[:, C_B:C_B + MT].sum())
    return np.float32(tot / float(N))


def kernel(inputs, targets, _trace=False, _tmpdir=None):
    X = np.asarray(inputs, dtype=np.float32)
    t = np.asarray(targets)
    assert X.shape == (N, D)

    if "nc" not in _CACHE:
        _CACHE["nc"] = _build_program()
    nc = _CACHE["nc"]

    in_maps = _prepare_in_maps(X, t)
    res = run_bass_kernel_spmd(
        nc, in_maps, list(range(NCORES)), trace=_trace, tmpdir=_tmpdir)
    loss = _reduce_outputs(res.results)
    if _trace:
        return loss, res
    return loss


# revision 40
# speedup vs baseline: 1.0535x; 1.0191x over previous
"""Trainium2 Bass kernel for nn_ContrastiveLoss (N=8192, D=1024, 751 ids).

loss = (1/N) * sum_ij [ same(i,j) & sim<1 -> (1-sim) ; diff(i,j) & sim>0.3 -> sim ]
with sim = X @ X.T.

Strategy (8 NeuronCores):
  * Host: sort rows by label (loss is permutation invariant); same-label
    pairs then live within +-63 of the diagonal (max class count <= 64).
    Quantize X to fp8 e4m3 -> PE runs DoubleRow matmuls (K=256/pass).
  * sim is symmetric: orient each unordered block pair {a,b} of the 16
    512-row blocks toward head a if (b-a) mod 16 in 1..8 (a<8) / 1..7.
    Core c takes heads c (9 items) and c+8 (8 items) -> 17 items/core,
    all 136 pairs covered once. Off-diagonal pairs weigh 2x.
  * Slot-uniform program: core c's DRAM holds the 16 blocks rotated by
    c (slot s = block (c+s) mod 16), so one program serves all cores.
    Each block is DMA'd once (fp8, 512KB) and stays resident in SBUF;
    items read lhs from slot 0/8, rhs from slot i -> ~8.5MB DMA/core.
  * Per item: 2 [128,1024] two-bank PSUM tiles (2 m-subtiles each),
    4 DoubleRow matmuls per subtile.  Unmasked sums need no label mask:
      sum_j sim*1[sim>0.3] = sum relu(sim-0.3) + 0.3*count(sim>0.3),
    ScalarE Relu with fused accum_out + VectorE is_gt with accum_out.
  * Band correction (circular 256-wide windows around the diagonal) on
    rows of blocks c and c+8 (slots 0/8): for same-label pairs subtract
    the neg term and add relu(1-sim), masked by a host-precomputed
    label-equality mask (saves the DVE is_equal); the 0.3*step term
    comes from a ScalarE Sign activation (saves the DVE is_gt).
    Windows slice the *same* resident tiles, so the sim values cancel
    bitwise; wrapped window columns are killed by the label mask.  Band
    items run in the first half so their DVE chains drain mid-kernel.
  * Junk warmup matmuls at t=0 open the HAM clock gate (2.4GHz) while
    the first block DMA lands.
  * Host: gather per-item partial sums, weight (1x diag / 2x off-diag),
    reduce in float64.
"""

import sys

for _p in ("/opt/trn_rl_repo",):
    if _p not in sys.path:
        sys.path.append(_p)

import numpy as np
import ml_dtypes

import concourse.bass as bass
import concourse.mybir as mybir
import concourse.tile as tile
from concourse import bacc
from concourse.bass_utils import run_bass_kernel_spmd

N = 8192           # rows
D = 1024           # feature dim
NCORES = 8
B = 512            # block size
NB = N // B        # 16 blocks/slots
NIT = 17           # items (block pairs) per core
MS = B // 128      # m-subtiles per item = 4
KT2 = D // 256     # DoubleRow contraction chunks = 4
MT = 8             # band row-tiles per core (2 groups x 4)
BW = 256           # band window width
MARGIN = 0.3

f8 = mybir.dt.float8e4
f16 = mybir.dt.float16
f32 = mybir.dt.float32

# output columns: per-half-item relu sums [0,34), counts [34,68),
# band corr [68,76), warmup junk [76]
MH = 2                  # m-subtile pairs per item ([128,1024] PSUM tiles)
C_R = 0
C_C = NIT * MH          # 34
C_B = 2 * NIT * MH      # 68
C_OUT = C_B + MT + 1    # 77

_CACHE = {}


def _band_segments(gi, j):
    """Window segments (slot, col0, width) for band row-tile j of group gi.

    Window = circular [S*512 + j*128 - 64, +256) in slot-column space.
    """
    S = 8 * gi
    if j == 0:
        return [((S - 1) % NB, 448, 64), (S, 0, 192)]
    if j == 3:
        return [(S, 320, 192), ((S + 1) % NB, 0, 64)]
    return [(S, j * 128 - 64, 256)]


def _build_program():
    nc = bacc.Bacc("TRN2", target_bir_lowering=False, debug=False,
                   num_devices=NCORES)

    # [slot, partition, (g,h), col]: slot s holds block (c+s)%16 of X^T,
    # k-row (2g+h)*128+p, quantized to fp8 e4m3.
    xq = nc.dram_tensor("xq", [NB, 128, 2 * KT2, B], f8, kind="ExternalInput")
    eqm = nc.dram_tensor("eqm", [128, MT * BW], f16, kind="ExternalInput")
    outp = nc.dram_tensor("out", [128, C_OUT], f32, kind="ExternalOutput")

    Relu = mybir.ActivationFunctionType.Relu
    Sign = mybir.ActivationFunctionType.Sign
    Op = mybir.AluOpType
    DR = mybir.MatmulPerfMode.DoubleRow

    with tile.TileContext(nc) as tc:
        with (
            tc.tile_pool(name="persist", bufs=1) as persist,
            tc.tile_pool(name="scr", bufs=3) as scr,
            tc.tile_pool(name="band", bufs=3) as bandp,
            tc.tile_pool(name="psum_m", bufs=3, space="PSUM") as psum_m,
            tc.tile_pool(name="psum_b", bufs=2, space="PSUM") as psum_b,
        ):
            T = [persist.tile([128, 2 * KT2, B], f8, name=f"blk{s}")
                 for s in range(NB)]
            eqm_sb = persist.tile([128, MT * BW], f16, name="eqm")
            stats = persist.tile([128, C_OUT], f32, name="stats")
            bias_m = persist.tile([128, 1], f32, name="bias_m")
            nc.vector.memset(bias_m[:], -MARGIN)

            # HAM warmup: junk matmuls keep the PE busy while the first
            # block DMA lands, so the clock gate opens before real work.
            wm = persist.tile([128, 2, 128], f8, name="wm")
            nc.vector.memset(wm[:], 0.125)
            wps = psum_m.tile([128, 2 * B], f32, name="mm")
            for w in range(24):
                nc.tensor.matmul(
                    wps[:, :128], wm[:], wm[:],
                    start=(w == 0), stop=(w == 23), perf_mode=DR)
            wsr = scr.tile([128, 128], f16, name="wsr")
            nc.scalar.activation(
                wsr[:], wps[:, :128], Relu,
                accum_out=stats[:, C_OUT - 1:C_OUT])

            def main_item(i):
                L = T[0] if i <= 8 else T[8]
                R = T[i] if i <= 8 else T[i - 1]
                for h in range(MH):
                    ps = psum_m.tile([128, 2 * B], f32, name="mm")
                    for m2 in range(2):
                        m = 2 * h + m2
                        pj = ps[:, m2 * B:(m2 + 1) * B]
                        for g in range(KT2):
                            nc.tensor.matmul(
                                pj,
                                L[:, 2 * g:2 * g + 2, m * 128:(m + 1) * 128],
                                R[:, 2 * g:2 * g + 2, :],
                                start=(g == 0), stop=(g == KT2 - 1),
                                perf_mode=DR,
                            )
                    col = i * MH + h
                    sr = scr.tile([128, 2 * B], f16, name="sr")
                    nc.scalar.activation(
                        sr[:], ps[:], Relu, bias=bias_m[:],
                        accum_out=stats[:, C_R + col:C_R + col + 1])
                    sc = scr.tile([128, 2 * B], f16, name="sc")
                    nc.vector.tensor_scalar(
                        sc[:], ps[:], MARGIN, None, op0=Op.is_gt,
                        op1=Op.add,
                        accum_out=stats[:, C_C + col:C_C + col + 1])

            def band_item(gi, j):
                """One [128 x 256] diagonal-window correction."""
                S = 8 * gi
                bj = gi * 4 + j
                ps = psum_b.tile([128, BW], f32, name="bb")
                off = 0
                for (sl, c0, w) in _band_segments(gi, j):
                    pj = ps[:, off:off + w]
                    for g in range(KT2):
                        nc.tensor.matmul(
                            pj,
                            T[S][:, 2 * g:2 * g + 2, j * 128:(j + 1) * 128],
                            T[sl][:, 2 * g:2 * g + 2, c0:c0 + w],
                            start=(g == 0), stop=(g == KT2 - 1),
                            perf_mode=DR,
                        )
                    off += w
                pos = bandp.tile([128, BW], f32, name="pos")
                rb = bandp.tile([128, BW], f32, name="rb")
                sg = bandp.tile([128, BW], f32, name="sg")
                # pos = relu(1 - s);  rb = relu(s - 0.3);  sg = sign(s - 0.3)
                # neg = rb + 0.3*1[s>0.3] = rb + 0.15*sg + 0.15  (a.e.)
                nc.scalar.activation(pos[:], ps[:], Relu, bias=1.0, scale=-1.0)
                nc.scalar.activation(rb[:], ps[:], Relu, bias=bias_m[:])
                nc.scalar.activation(sg[:], ps[:], Sign, bias=bias_m[:])
                # corr (negated) = eq * (neg - pos); host subtracts it
                a = bandp.tile([128, BW], f32, name="a")
                nc.vector.scalar_tensor_tensor(
                    a[:], sg[:], 0.15, pos[:], op0=Op.mult,
                    op1=Op.subtract)
                b = bandp.tile([128, BW], f32, name="b")
                nc.vector.tensor_tensor(b[:], a[:], rb[:], op=Op.add)
                crr = bandp.tile([128, BW], f32, name="crr")
                nc.vector.scalar_tensor_tensor(
                    crr[:], b[:], 0.15, eqm_sb[:, bj * BW:(bj + 1) * BW],
                    op0=Op.add, op1=Op.mult,
                    accum_out=stats[:, C_B + bj:C_B + bj + 1])

            # DMA slot order: band-needed slots (15, 7, 8) pulled early so
            # all band items can run in the first half; mains stay fed.
            dma_order = [0, 1, 2, 15, 3, 7, 8, 4, 5, 9, 6, 10, 11, 12, 13, 14]
            # band items early so their DVE chains drain mid-kernel, not
            # as a serialized tail behind the last main items
            bands_after = {
                0: [(0, 1)], 1: [(0, 2)], 2: [(0, 0)], 3: [(0, 3)],
                4: [(1, 0)], 5: [(1, 1)], 6: [(1, 2)], 7: [(1, 3)],
            }

            # T0 split in half so the first matmuls start sooner
            nc.sync.dma_start(T[0][:, 0:KT2, :], xq[0][:, 0:KT2, :])
            nc.sync.dma_start(T[0][:, KT2:, :], xq[0][:, KT2:, :])
            for p in range(1, 4):
                nc.sync.dma_start(T[dma_order[p]][:], xq[dma_order[p]])
            nc.sync.dma_start(eqm_sb[:], eqm[:])
            for i in range(NIT):
                for p in (2 * i + 4, 2 * i + 5):
                    if p < NB:
                        nc.sync.dma_start(T[dma_order[p]][:],
                                          xq[dma_order[p]])
                main_item(i)
                for (gi, j) in bands_after.get(i, ()):
                    band_item(gi, j)

            nc.sync.dma_start(outp[:], stats[:])

    nc.compile()
    return nc


def _prepare_in_maps(X, t):
    perm = np.argsort(t, kind="stable")
    Xs = X[perm]
    ts = t[perm].astype(np.int64)
    counts = np.bincount(ts)
    maxc = int(counts.max()) if counts.size else 0
    assert maxc <= 64, f"class count {maxc} exceeds band half-width 64"
    XT = np.ascontiguousarray(Xs.astype(ml_dtypes.float8_e4m3).T)  # [D, N]
    # [b, p, (g,h), col]
    blocks = np.ascontiguousarray(
        XT.reshape(KT2, 2, 128, NB, B).transpose(3, 2, 0, 1, 4)
    ).reshape(NB, 128, 2 * KT2, B)
    tsf = ts.astype(np.float16)  # exact for ids < 2048

    in_maps = []
    for c in range(NCORES):
        order = [(c + s) % NB for s in range(NB)]
        xqc = np.ascontiguousarray(blocks[order])
        eq = np.empty((128, MT * BW), np.float16)
        for gi in range(2):
            base = (c + 8 * gi) % NB
            for j in range(4):
                bj = gi * 4 + j
                r0 = base * B + j * 128
                idx = (np.arange(BW) + r0 - 64) % N
                eq[:, bj * BW:(bj + 1) * BW] = (
                    ts[r0:r0 + 128, None] == ts[idx][None, :])
        in_maps.append({"xq": xqc, "eqm": eq})
    return in_maps


# item weights: diag items (slots 0 and 8 vs themselves) 1x, rest 2x
_W_ITEM = np.array([1.0] + [2.0] * 8 + [1.0] + [2.0] * 7, np.float64)


def _reduce_outputs(results):
    tot = 0.0
    for c in range(NCORES):
        o = np.asarray(results[c]["out"], np.float64)
        r_items = o[:, C_R:C_C].sum(axis=0).reshape(NIT, MH).sum(axis=1)
        c_items = o[:, C_C:C_B].sum(axis=0).reshape(NIT, MH).sum(axis=1)
        neg_items = r_items + MARGIN * c_items
        tot += float((_W_ITEM * neg_items).sum())
        # band cols hold eq*(neg - pos); subtract
        tot -= float(o[:, C_B:C_B + MT].sum())
    return np.float32(tot / float(N))


def kernel(inputs, targets, _trace=False, _tmpdir=None):
    X = np.asarray(inputs, dtype=np.float32)
    t = np.asarray(targets)
    assert X.shape == (N, D)

    if "nc" not in _CACHE:
        _CACHE["nc"] = _build_program()
    nc = _CACHE["nc"]

    in_maps = _prepare_in_maps(X, t)
    res = run_bass_kernel_spmd(
        nc, in_maps, list(range(NCORES)), trace=_trace, tmpdir=_tmpdir)
    loss = _reduce_outputs(res.results)
    if _trace:
        return loss, res
    return loss
